# revision 1
# baseline (speedup 1.0000x reference)
"""LDA loss (inter/intra hinge) on 8 Trainium2 NeuronCores.

Strategy (data-parallel over B, hence over G=B/16 centers):
  Launch 1 (per core, local shard of 16384 samples):
    - centers via PE selector matmul (groups of 16 samples)
    - diff = x - c(group) via PE accumulation (identity MM + -selector MM)
    - intra partials: d2 = reduce(diff^2), hinge tail on [128,128]
    - exports packed local centers
  Host: gather centers, cast bf16, rotate per core so each core's own
    1024-center block is first (=> its diagonal block of the GxG matrix
    sits at a static column offset), compute center norms.
  Launch 2 (per core, block-row of the GxG pairwise matrix):
    - psum = -2*C_loc^T C_all + ||c_j||^2 (K=1 ones matmul) + 512*I on
      the diagonal block (identity matmul diag fix => hinge exactly 0)
    - ACT: t = sqrt(psum + ||c_i||^2 + eps)   (bias per partition)
    - DVE: w = min(t,1)-1 ;  sum w^2 via tensor_tensor_reduce
  Host: inter = sum/2/n_pairs, intra = sum/(G*16).

Exact-zero property: every off-diagonal pair with d^2 >= 1 yields
min(t,1) = 1 exactly => w = 0 exactly => the all-zero inter loss of the
reference is reproduced bit-exactly (0.0).
"""
import sys

if "/opt/trn_rl_repo" not in sys.path:
    sys.path.insert(0, "/opt/trn_rl_repo")

import numpy as np
import ml_dtypes

import concourse.bacc as bacc
import concourse.tile as tile
from concourse import mybir
from concourse.bass_utils import run_bass_kernel_spmd

N_CORES = 8
B, D, P = 131072, 128, 16
G = B // P                 # 8192 centers
GL = G // N_CORES          # 1024 local centers
SL = B // N_CORES          # 16384 local samples
NT = SL // 128             # 128 sample tiles / core
EPS = 1e-3
BIG = 512.0

F32 = mybir.dt.float32
BF16 = mybir.dt.bfloat16

_cache = {}
_last_traces = {}


def _build_launch1():
    nc = bacc.Bacc("TRN2", target_bir_lowering=False, debug=False,
                   num_devices=N_CORES)
    fea = nc.dram_tensor("fea", [SL, D], F32, kind="ExternalInput").ap()
    sel = nc.dram_tensor("sel", [128, 8], F32, kind="ExternalInput").ap()
    ident = nc.dram_tensor("ident", [128, 128], F32, kind="ExternalInput").ap()
    negE = nc.dram_tensor("negE", [8, 128], F32, kind="ExternalInput").ap()
    cpack = nc.dram_tensor("cpack", [8, SL], F32, kind="ExternalOutput").ap()
    ipart = nc.dram_tensor("ipart", [128, 1], F32, kind="ExternalOutput").ap()

    fea3 = fea.rearrange("(b p) d -> p b d", p=128)  # [128, NT, 128]

    with tile.TileContext(nc) as tc:
        with (
            tc.tile_pool(name="persist", bufs=1) as pp,
            tc.tile_pool(name="small", bufs=1) as sp,
            tc.tile_pool(name="ps1", bufs=2, space="PSUM") as psp1,
            tc.tile_pool(name="d2p", bufs=2) as d2pool,
        ):
            t_fea = pp.tile([128, SL], F32, tag="fea")
            tf3 = t_fea[:].rearrange("p (b d) -> p b d", d=128)
            for k in range(16):
                nc.sync.dma_start(tf3[:, 8 * k:8 * (k + 1), :],
                                  fea3[:, 8 * k:8 * (k + 1), :])
            t_sel = sp.tile([128, 8], F32, tag="sel")
            nc.sync.dma_start(t_sel[:], sel[:])
            t_id = sp.tile([128, 128], F32, tag="id")
            nc.sync.dma_start(t_id[:], ident[:])
            t_ne = sp.tile([8, 128], F32, tag="ne")
            nc.sync.dma_start(t_ne[:], negE[:])

            t_cpack = pp.tile([8, SL], F32, tag="cpack")

            # ---- centers: cpack[q, 2048 r + 128 slot + d] = c[8(16r+slot)+q, d]
            for r in range(8):
                cps = psp1.tile([128, 2048], F32, tag="ps1")
                for c in range(4):
                    nc.tensor.matmul(
                        cps[:8, 512 * c:512 * (c + 1)], t_sel[:, :],
                        t_fea[:, 2048 * r + 512 * c:2048 * r + 512 * (c + 1)],
                        start=True, stop=True)
                nc.scalar.copy(t_cpack[:, 2048 * r:2048 * (r + 1)], cps[:8, :])
            for q in range(8):
                nc.sync.dma_start(cpack[q:q + 1, :], t_cpack[q:q + 1, :])

            # ---- intra: diff in PSUM via I*x - E*c, then d2 = sum(diff^2)
            t_d2 = pp.tile([128, 128], F32, tag="d2")
            for r in range(8):
                dps = psp1.tile([128, 2048], F32, tag="ps1")
                for c in range(4):
                    nc.tensor.matmul(
                        dps[:, 512 * c:512 * (c + 1)], t_id[:, :],
                        t_fea[:, 2048 * r + 512 * c:2048 * r + 512 * (c + 1)],
                        start=True, stop=False)
                for c in range(4):
                    nc.tensor.matmul(
                        dps[:, 512 * c:512 * (c + 1)], t_ne[:, :],
                        t_cpack[:8, 2048 * r + 512 * c:2048 * r + 512 * (c + 1)],
                        start=False, stop=True)
                d2sq = d2pool.tile([128, 2048], F32, tag="d2sq")
                nc.scalar.activation(d2sq[:], dps[:],
                                     mybir.ActivationFunctionType.Square)
                nc.vector.tensor_reduce(
                    t_d2[:, 16 * r:16 * (r + 1)],
                    d2sq[:].rearrange("p (t d) -> p t d", d=128),
                    axis=mybir.AxisListType.X, op=mybir.AluOpType.add)

            # ---- hinge tail on [128, 128]
            t_di = sp.tile([128, 128], F32, tag="di")
            nc.scalar.activation(t_di[:], t_d2[:],
                                 mybir.ActivationFunctionType.Sqrt)
            t_w = sp.tile([128, 128], F32, tag="w")
            nc.vector.tensor_scalar(t_w[:], t_di[:], 0.1, 0.0,
                                    op0=mybir.AluOpType.subtract,
                                    op1=mybir.AluOpType.max)
            t_w2 = sp.tile([128, 128], F32, tag="w2")
            t_acc = sp.tile([128, 1], F32, tag="acc")
            nc.scalar.activation(t_w2[:], t_w[:],
                                 mybir.ActivationFunctionType.Square,
                                 accum_out=t_acc[:])
            nc.sync.dma_start(ipart[:], t_acc[:])
    nc.compile()
    return nc


def _build_launch2():
    nc = bacc.Bacc("TRN2", target_bir_lowering=False, debug=False,
                   num_devices=N_CORES)
    ctr = nc.dram_tensor("ctr", [128, G], BF16, kind="ExternalInput").ap()
    lhsTloc = nc.dram_tensor("lhsTloc", [128, GL], BF16,
                             kind="ExternalInput").ap()
    sqrow = nc.dram_tensor("sqrow", [2, G], BF16, kind="ExternalInput").ap()
    sqbias = nc.dram_tensor("sqbias", [128, 8], F32, kind="ExternalInput").ap()
    idb512 = nc.dram_tensor("idb512", [128, 128], BF16,
                            kind="ExternalInput").ap()
    idbI = nc.dram_tensor("idbI", [128, 128], BF16, kind="ExternalInput").ap()
    ones1 = nc.dram_tensor("ones1", [2, 128], BF16, kind="ExternalInput").ap()
    inter_p = nc.dram_tensor("inter_p", [128, 1], F32,
                             kind="ExternalOutput").ap()

    with tile.TileContext(nc) as tc:
        with (
            tc.tile_pool(name="persist", bufs=1) as pp,
            tc.tile_pool(name="work", bufs=3) as wp,
            tc.tile_pool(name="ps", bufs=2, space="PSUM") as psp,
        ):
            t_ctr = pp.tile([128, G], BF16, tag="ctr")
            for k in range(8):
                nc.sync.dma_start(t_ctr[:, 1024 * k:1024 * (k + 1)],
                                  ctr[:, 1024 * k:1024 * (k + 1)])
            t_lh = pp.tile([128, GL], BF16, tag="lh")
            nc.sync.dma_start(t_lh[:], lhsTloc[:])
            t_sq = pp.tile([2, G], BF16, tag="sq")
            nc.sync.dma_start(t_sq[:], sqrow[:])
            t_sb = pp.tile([128, 8], F32, tag="sb")
            nc.sync.dma_start(t_sb[:], sqbias[:])
            t_i5 = pp.tile([128, 128], BF16, tag="i5")
            nc.sync.dma_start(t_i5[:], idb512[:])
            t_ii = pp.tile([128, 128], BF16, tag="ii")
            nc.sync.dma_start(t_ii[:], idbI[:])
            t_o1 = pp.tile([2, 128], BF16, tag="o1")
            nc.sync.dma_start(t_o1[:], ones1[:])

            t_accs = pp.tile([128, 32], F32, tag="accs")

            for m in range(8):
                for q in range(4):
                    pt = psp.tile([128, 2048], F32, tag="pt")
                    for c in range(4):
                        nc.tensor.matmul(
                            pt[:, 512 * c:512 * (c + 1)],
                            t_lh[:, 128 * m:128 * (m + 1)],
                            t_ctr[:, 2048 * q + 512 * c:2048 * q + 512 * (c + 1)],
                            start=True, stop=False)
                    for c in range(4):
                        nc.tensor.matmul(
                            pt[:, 512 * c:512 * (c + 1)],
                            t_o1[:2, :],
                            t_sq[:2, 2048 * q + 512 * c:2048 * q + 512 * (c + 1)],
                            start=False, stop=True)
                    if q == 0:
                        # diagonal block: add 512*I so hinge is exactly 0
                        nc.tensor.matmul(pt[:, 128 * m:128 * (m + 1)],
                                         t_i5[:, :], t_ii[:, :],
                                         start=False, stop=True,
                                         skip_group_check=True)
                    th = wp.tile([128, 2048], BF16, tag="th")
                    nc.scalar.activation(th[:], pt[:],
                                         mybir.ActivationFunctionType.Sqrt,
                                         bias=t_sb[:, m:m + 1], scale=1.0)
                    tw = wp.tile([128, 2048], BF16, tag="tw")
                    nc.vector.tensor_scalar(tw[:], th[:], 1.0, 1.0,
                                            op0=mybir.AluOpType.min,
                                            op1=mybir.AluOpType.subtract)
                    col = t_accs[:, 4 * m + q:4 * m + q + 1]
                    if (m + q) % 2 == 0:
                        tw2 = wp.tile([128, 2048], BF16, tag="tw2")
                        nc.scalar.activation(tw2[:], tw[:],
                                             mybir.ActivationFunctionType.Square,
                                             accum_out=col)
                    else:
                        tw2 = wp.tile([128, 2048], BF16, tag="tw2")
                        nc.vector.tensor_mul(tw2[:], tw[:], tw[:])
                        nc.vector.tensor_reduce(col, tw2[:],
                                                axis=mybir.AxisListType.X,
                                                op=mybir.AluOpType.add)

            t_tot = pp.tile([128, 1], F32, tag="tot")
            nc.vector.tensor_reduce(t_tot[:], t_accs[:],
                                    axis=mybir.AxisListType.X,
                                    op=mybir.AluOpType.add)
            nc.sync.dma_start(inter_p[:], t_tot[:])
    nc.compile()
    return nc


def _get(name, builder):
    if name not in _cache:
        _cache[name] = builder()
    return _cache[name]


def kernel(path_fea):
    fea = np.ascontiguousarray(
        np.asarray(path_fea, dtype=np.float32).reshape(B, D))

    trace = bool(int(__import__("os").environ.get("KERNEL_TRACE", "0")))
    runkw = {}
    if trace:
        import trace_shim
        trace_shim.install()
        runkw = dict(trace=True)

    # ---------------- launch 1 ----------------
    nc1 = _get("l1", _build_launch1)
    sel = np.zeros((128, 8), np.float32)
    for s in range(128):
        sel[s, s // 16] = 1.0 / 16.0
    negE = np.zeros((8, 128), np.float32)
    for s in range(128):
        negE[s // 16, s] = -1.0
    ident = np.eye(128, dtype=np.float32)
    in1 = [{"fea": fea[SL * c:SL * (c + 1)], "sel": sel, "ident": ident,
            "negE": negE} for c in range(N_CORES)]
    r1 = run_bass_kernel_spmd(nc1, in1, core_ids=list(range(N_CORES)), **runkw)
    if trace and r1.exec_time_ns is not None:
        print(f"[launch1] HW exec time: {r1.exec_time_ns} ns")
        _last_traces["launch1"] = r1

    # ---------------- host gather ----------------
    centers = np.empty((G, D), np.float32)
    ipart_sum = 0.0
    for c in range(N_CORES):
        cp = r1.results[c]["cpack"].reshape(8, 8, 16, 128)     # q r slot d
        centers[GL * c:GL * (c + 1)] = (
            cp.transpose(1, 2, 0, 3).reshape(GL, 128))
        ipart_sum += float(r1.results[c]["ipart"].astype(np.float64).sum())

    cbf = centers.astype(ml_dtypes.bfloat16)
    sq = (cbf.astype(np.float32) ** 2).sum(1)                   # [G] f32

    idb512 = (BIG * np.eye(128)).astype(ml_dtypes.bfloat16)
    idbI = np.eye(128, dtype=np.float32).astype(ml_dtypes.bfloat16)
    ones1 = np.ones((2, 128), np.float32).astype(ml_dtypes.bfloat16)
    in2 = []
    for c in range(N_CORES):
        idx = (np.arange(G) + GL * c) % G
        ctr_rot = np.ascontiguousarray(cbf[idx].T)              # [128, G]
        sq_r = sq[idx]
        hi = sq_r.astype(ml_dtypes.bfloat16)
        lo = (sq_r - hi.astype(np.float32)).astype(ml_dtypes.bfloat16)
        sqrow = np.ascontiguousarray(np.stack([hi, lo]))
        loc = cbf[GL * c:GL * (c + 1)].astype(np.float32) * -2.0
        lhsTloc = np.ascontiguousarray(loc.astype(ml_dtypes.bfloat16).T)
        sqbias = np.ascontiguousarray(
            (sq[GL * c:GL * (c + 1)] + EPS).reshape(8, 128).T)
        in2.append({"ctr": ctr_rot, "lhsTloc": lhsTloc, "sqrow": sqrow,
                    "sqbias": sqbias, "idb512": idb512, "idbI": idbI,
                    "ones1": ones1})

    nc2 = _get("l2", _build_launch2)
    r2 = run_bass_kernel_spmd(nc2, in2, core_ids=list(range(N_CORES)), **runkw)
    if trace and r2.exec_time_ns is not None:
        print(f"[launch2] HW exec time: {r2.exec_time_ns} ns")
        _last_traces["launch2"] = r2

    inter_sum = 0.0
    for c in range(N_CORES):
        inter_sum += float(r2.results[c]["inter_p"].astype(np.float64).sum())

    n_pairs = G * (G - 1) / 2.0
    inter = np.float32(inter_sum / 2.0 / n_pairs)
    intra = np.float32(ipart_sum / (G * P))
    return (inter, intra)



# revision 3
# speedup vs baseline: 1.5424x; 1.5424x over previous
"""LDA loss (inter/intra hinge) on 8 Trainium2 NeuronCores.

Strategy (data-parallel over B; G=B/16 centers; all-bf16 data path):

  Host staging: cast path_fea to bf16, rearrange each core's shard to
    p-major [128, b, d] so the device load is fully contiguous.

  Launch 1 (per core, 16384 samples):
    - centers via PE selector matmul ([8, 2048] psum per 16-tile chunk)
    - ACT casts centers to bf16 (export + reuse)
    - DMA replicates center rows across partition groups (16x)
    - DVE diff = x - c, ACT square, DVE segmented reduce -> d2 [128,128]
    - tiny f32 hinge tail -> ipart [128, 1]

  Host: gather centers (bf16), build per-core rotated center panels.

  Launch 2 (per core, cyclic-half of the GxG pairwise matrix):
    Uniform triangle: 16 row-chunks of 512; core c owns chunks c and c+8.
    Each row-chunk processes 9 column blocks (its own + next 8 mod 16)
    from a rotated+extended center panel [128, 8704]:
      psum = -2*C_loc^T C_ext + (ones^T [hi;lo]) (K=2 matmul adds ||c_j||^2)
      ACT: t = sqrt(psum + ||c_i||^2 + eps)  (bias per partition)
      DVE: w = min(t,1)-1 ; sum w^2 via scalar_tensor_tensor accum
    Separate accumulators for diag block (self pairs + double-counted),
    middle blocks (counted once), and far block (double-counted).
  Host: inter = (S_mid + (S_diag - S_self)/2 + S_far/2) / n_pairs.

Exact-zero property: every pair with d >= 1 yields min(t,1)-1 = 0
exactly, so the all-zero inter loss is reproduced up to the tiny
host-side self-pair correction (~1e-7 absolute).
"""
import sys

if "/opt/trn_rl_repo" not in sys.path:
    sys.path.insert(0, "/opt/trn_rl_repo")

import numpy as np
import ml_dtypes

import concourse.bacc as bacc
import concourse.tile as tile
from concourse import mybir
from concourse.bass_utils import run_bass_kernel_spmd

N_CORES = 8
B, D, P = 131072, 128, 16
G = B // P                 # 8192 centers
GL = G // N_CORES          # 1024 local centers (rows) per core
SL = B // N_CORES          # 16384 local samples
NT = SL // 128             # 128 sample tiles / core
CW = 512                   # row-chunk width (16 chunks globally)
EXT = 17 * CW              # 8704 extended column panel
EPS = 1e-3
MI = 0.1

F32 = mybir.dt.float32
BF16 = mybir.dt.bfloat16
AF = mybir.ActivationFunctionType
OP = mybir.AluOpType
AX = mybir.AxisListType

_cache = {}
_last_traces = {}


def _build_launch1():
    nc = bacc.Bacc("TRN2", target_bir_lowering=False, debug=False,
                   num_devices=N_CORES)
    xp = nc.dram_tensor("xp", [128, SL], BF16, kind="ExternalInput").ap()
    sel = nc.dram_tensor("sel", [128, 8], BF16, kind="ExternalInput").ap()
    cpack = nc.dram_tensor("cpack", [8, SL], BF16, kind="ExternalOutput").ap()
    ipart = nc.dram_tensor("ipart", [128, 1], F32, kind="ExternalOutput").ap()

    with tile.TileContext(nc) as tc:
        with (
            tc.tile_pool(name="persist", bufs=1) as pp,
            tc.tile_pool(name="work", bufs=3) as wp,
            tc.tile_pool(name="ps1", bufs=2, space="PSUM") as psp,
        ):
            t_xp = pp.tile([128, SL], BF16, tag="xp")
            for k in range(8):
                nc.sync.dma_start(t_xp[:, 2048 * k:2048 * (k + 1)],
                                  xp[:, 2048 * k:2048 * (k + 1)])
            t_sel = pp.tile([128, 8], BF16, tag="sel")
            nc.sync.dma_start(t_sel[:], sel[:])
            t_ct = pp.tile([128, SL], BF16, tag="ct")
            t_d2 = pp.tile([128, NT], F32, tag="d2")

            for k in range(8):
                sl2 = slice(2048 * k, 2048 * (k + 1))
                cps = psp.tile([128, 2048], F32, tag="cps")
                for j in range(4):
                    nc.tensor.matmul(
                        cps[:8, 512 * j:512 * (j + 1)], t_sel[:, :],
                        t_xp[:, 2048 * k + 512 * j:2048 * k + 512 * (j + 1)],
                        start=True, stop=True)
                cseg = wp.tile([8, 2048], BF16, tag="cseg")
                nc.scalar.copy(cseg[:], cps[:8, :])
                nc.sync.dma_start(cpack[:, sl2], cseg[:])
                # replicate each center row across its 16 sample partitions
                for j in range(16):
                    nc.sync.dma_start(t_ct[j::16, sl2], cseg[:, :])
                diff = wp.tile([128, 2048], BF16, tag="diff")
                nc.vector.tensor_tensor(diff[:], t_xp[:, sl2], t_ct[:, sl2],
                                        op=OP.subtract)
                dsq = wp.tile([128, 2048], F32, tag="dsq")
                nc.scalar.activation(dsq[:], diff[:], AF.Square)
                nc.vector.tensor_reduce(
                    t_d2[:, 16 * k:16 * (k + 1)],
                    dsq[:].rearrange("p (b d) -> p b d", d=128),
                    axis=AX.X, op=OP.add)

            t_di = pp.tile([128, NT], F32, tag="di")
            nc.scalar.activation(t_di[:], t_d2[:], AF.Sqrt)
            t_w = pp.tile([128, NT], F32, tag="w")
            nc.vector.tensor_scalar(t_w[:], t_di[:], MI, 0.0,
                                    op0=OP.subtract, op1=OP.max)
            t_w2 = pp.tile([128, NT], F32, tag="w2")
            t_acc = pp.tile([128, 1], F32, tag="acc")
            nc.vector.scalar_tensor_tensor(t_w2[:], t_w[:], 0.0, t_w[:],
                                           op0=OP.bypass, op1=OP.mult,
                                           accum_out=t_acc[:])
            nc.sync.dma_start(ipart[:], t_acc[:])
    nc.compile()
    return nc


def _build_launch2():
    nc = bacc.Bacc("TRN2", target_bir_lowering=False, debug=False,
                   num_devices=N_CORES)
    ctr = nc.dram_tensor("ctr", [128, EXT], BF16, kind="ExternalInput").ap()
    lh = nc.dram_tensor("lh", [128, GL], BF16, kind="ExternalInput").ap()
    sqrow = nc.dram_tensor("sqrow", [2, EXT], BF16, kind="ExternalInput").ap()
    sqbias = nc.dram_tensor("sqbias", [128, 8], F32, kind="ExternalInput").ap()
    ones1 = nc.dram_tensor("ones1", [2, 128], BF16, kind="ExternalInput").ap()
    accs = nc.dram_tensor("accs", [128, 32], F32, kind="ExternalOutput").ap()

    with tile.TileContext(nc) as tc:
        with (
            tc.tile_pool(name="persist", bufs=1) as pp,
            tc.tile_pool(name="work", bufs=3) as wp,
            tc.tile_pool(name="ps", bufs=2, space="PSUM") as psp,
        ):
            t_ctr = pp.tile([128, EXT], BF16, tag="ctr")
            for k in range(4):
                nc.sync.dma_start(t_ctr[:, 2048 * k:2048 * (k + 1)],
                                  ctr[:, 2048 * k:2048 * (k + 1)])
            nc.sync.dma_start(t_ctr[:, 8192:EXT], ctr[:, 8192:EXT])
            t_lh = pp.tile([128, GL], BF16, tag="lh")
            nc.sync.dma_start(t_lh[:], lh[:])
            t_sq = pp.tile([2, EXT], BF16, tag="sq")
            nc.sync.dma_start(t_sq[:], sqrow[:])
            t_sb = pp.tile([128, 8], F32, tag="sb")
            nc.sync.dma_start(t_sb[:], sqbias[:])
            t_o1 = pp.tile([2, 128], BF16, tag="o1")
            nc.sync.dma_start(t_o1[:], ones1[:])

            t_accs = pp.tile([128, 32], F32, tag="accs")

            # col tiles per m: [lo, lo+2048), [lo+2048, lo+4096), [lo+4096, lo+4608)
            for m in range(8):
                base = 0 if m < 4 else 4096
                for t, (c0, cw) in enumerate([(0, 2048), (2048, 2048),
                                              (4096, 512)]):
                    lo = base + c0
                    pt = psp.tile([128, 2048], F32, tag="pt")
                    nmm = cw // 512
                    for j in range(nmm):
                        nc.tensor.matmul(
                            pt[:, 512 * j:512 * (j + 1)],
                            t_lh[:, 128 * m:128 * (m + 1)],
                            t_ctr[:, lo + 512 * j:lo + 512 * (j + 1)],
                            start=True, stop=False)
                    for j in range(nmm):
                        nc.tensor.matmul(
                            pt[:, 512 * j:512 * (j + 1)],
                            t_o1[:2, :],
                            t_sq[:2, lo + 512 * j:lo + 512 * (j + 1)],
                            start=False, stop=True)
                    th = wp.tile([128, 2048], BF16, tag="th")
                    nc.scalar.activation(th[:, :cw], pt[:, :cw], AF.Sqrt,
                                         bias=t_sb[:, m:m + 1], scale=1.0)
                    tw = wp.tile([128, 2048], BF16, tag="tw")
                    nc.vector.tensor_scalar(tw[:, :cw], th[:, :cw], 1.0, 1.0,
                                            op0=OP.min, op1=OP.subtract)
                    tw2 = wp.tile([128, 2048], BF16, tag="tw2")
                    if t == 0:
                        nc.vector.scalar_tensor_tensor(
                            tw2[:, :512], tw[:, :512], 0.0, tw[:, :512],
                            op0=OP.bypass, op1=OP.mult,
                            accum_out=t_accs[:, 4 * m:4 * m + 1])
                        nc.vector.scalar_tensor_tensor(
                            tw2[:, 512:2048], tw[:, 512:2048], 0.0,
                            tw[:, 512:2048],
                            op0=OP.bypass, op1=OP.mult,
                            accum_out=t_accs[:, 4 * m + 1:4 * m + 2])
                    elif t == 1:
                        nc.vector.scalar_tensor_tensor(
                            tw2[:, :2048], tw[:, :2048], 0.0, tw[:, :2048],
                            op0=OP.bypass, op1=OP.mult,
                            accum_out=t_accs[:, 4 * m + 2:4 * m + 3])
                    else:
                        nc.vector.scalar_tensor_tensor(
                            tw2[:, :512], tw[:, :512], 0.0, tw[:, :512],
                            op0=OP.bypass, op1=OP.mult,
                            accum_out=t_accs[:, 4 * m + 3:4 * m + 4])
            nc.sync.dma_start(accs[:], t_accs[:])
    nc.compile()
    return nc


def _get(name, builder):
    if name not in _cache:
        _cache[name] = builder()
    return _cache[name]


def kernel(path_fea):
    fea = np.asarray(path_fea, dtype=np.float32).reshape(B, D)

    trace = bool(int(__import__("os").environ.get("KERNEL_TRACE", "0")))
    runkw = {}
    if trace:
        import trace_shim
        trace_shim.install()
        runkw = dict(trace=True)

    # ---------------- launch 1 ----------------
    nc1 = _get("l1", _build_launch1)
    xbf = fea.astype(ml_dtypes.bfloat16)
    sel = np.zeros((128, 8), np.float32)
    for s in range(128):
        sel[s, s // 16] = 1.0 / 16.0
    sel = sel.astype(ml_dtypes.bfloat16)
    in1 = []
    for c in range(N_CORES):
        sh = xbf[SL * c:SL * (c + 1)].reshape(NT, 128, D).transpose(1, 0, 2)
        in1.append({"xp": np.ascontiguousarray(sh.reshape(128, SL)),
                    "sel": sel})
    r1 = run_bass_kernel_spmd(nc1, in1, core_ids=list(range(N_CORES)), **runkw)
    if trace and r1.exec_time_ns is not None:
        print(f"[launch1] HW exec time: {r1.exec_time_ns} ns")
        _last_traces["launch1"] = r1

    # ---------------- host gather ----------------
    centers = np.empty((G, D), ml_dtypes.bfloat16)
    ipart_sum = 0.0
    for c in range(N_CORES):
        cp = r1.results[c]["cpack"].reshape(8, NT, D)      # slot b d
        centers[GL * c:GL * (c + 1)] = cp.transpose(1, 0, 2).reshape(GL, D)
        ipart_sum += float(r1.results[c]["ipart"].astype(np.float64).sum())

    cf = centers.astype(np.float32)
    sq = (cf ** 2).sum(1)                                  # [G] f32
    hi = sq.astype(ml_dtypes.bfloat16)
    lo = (sq - hi.astype(np.float32)).astype(ml_dtypes.bfloat16)
    ctrT = np.ascontiguousarray(centers.T)                 # [128, G] bf16
    ones1 = np.ones((2, 128), np.float32).astype(ml_dtypes.bfloat16)

    in2 = []
    rowsets = []
    for c in range(N_CORES):
        idx = (np.arange(EXT) + CW * c) % G
        ctr_ext = np.ascontiguousarray(ctrT[:, idx])
        sqrow = np.ascontiguousarray(np.stack([hi[idx], lo[idx]]))
        rows = np.concatenate([np.arange(CW * c, CW * (c + 1)),
                               np.arange(CW * (c + 8), CW * (c + 9))])
        rowsets.append(rows)
        lhT = np.ascontiguousarray(
            (cf[rows].T * np.float32(-2.0)).astype(ml_dtypes.bfloat16))
        sqb = np.ascontiguousarray(
            (sq[rows] + np.float32(EPS)).reshape(8, 128).T)
        in2.append({"ctr": ctr_ext, "lh": lhT, "sqrow": sqrow,
                    "sqbias": sqb, "ones1": ones1})

    nc2 = _get("l2", _build_launch2)
    r2 = run_bass_kernel_spmd(nc2, in2, core_ids=list(range(N_CORES)), **runkw)
    if trace and r2.exec_time_ns is not None:
        print(f"[launch2] HW exec time: {r2.exec_time_ns} ns")
        _last_traces["launch2"] = r2

    # ---------------- host reduce ----------------
    S_diag = S_mid = S_far = 0.0
    for c in range(N_CORES):
        a = r2.results[c]["accs"].astype(np.float64)       # [128, 32]
        for m in range(8):
            S_diag += a[:, 4 * m].sum()
            S_mid += a[:, 4 * m + 1].sum() + a[:, 4 * m + 2].sum()
            S_far += a[:, 4 * m + 3].sum()

    # host model of the device's self-pair terms (d^2 ~ 0 -> t ~ sqrt(eps))
    S_self = 0.0
    f32 = np.float32
    for c in range(N_CORES):
        rows = rowsets[c]
        s = sq[rows]
        h = hi[rows].astype(np.float32)
        l = lo[rows].astype(np.float32)
        arg = (f32(-2.0) * s + (h + l)) + (s + f32(EPS))
        t = np.sqrt(np.maximum(arg, 0.0))
        w = np.minimum(t, 1.0) - 1.0
        S_self += float((w.astype(np.float64) ** 2).sum())

    n_pairs = G * (G - 1) / 2.0
    inter_sum = S_mid + (S_diag - S_self) / 2.0 + S_far / 2.0
    inter = np.float32(inter_sum / n_pairs)
    intra = np.float32(ipart_sum / (G * P))
    return (inter, intra)
